# revision 1
# baseline (speedup 1.0000x reference)
"""MBConv (4D spatial, 16^4) on 8 TRN2 NeuronCores.

Sharding: spatial-parallel over the first spatial dim X (16 planes ->
2 owned planes per core + 1 halo plane each side, shipped from host).

Math (all on device except weight-only constant folding on host):
  GN0+conv1+GN1 folded: A' = (W1 * g0_w) . x computed once; the two
  global groupnorms reduce to 6 scalars in ONE AllReduce:
    [Sum(A'), Sum(A'^2), Sum(u*SA), Sum(v*SA), Sum(x), Sum(x^2)]
  with u = W1.g0_b, v = W1.g0_w (host constants); then
  h1 = gelu(alpha1 * A' + beta1) per hidden channel.
  conv2 = 81 accumulating PE matmuls per PSUM bank over a zero-padded
  [128ch, 4planes, 18,18,18] SBUF tile (bf16).
  GN2 -> AllReduce(2 scalars); gelu fused with SE mean via accum_out.
  SE mean -> AllReduce(128); SE MLP on-device; scale folded into w3.
  conv3 (bf16); GN3 -> AllReduce(2 scalars); affine; DMA out.

v2 perf structure:
  - warmup AllReduce at t=0 absorbs CC-engine startup + core skew
  - x/conv1/conv3 in bf16 (fp32r matmuls run ~2x slow on HW)
  - conv1 owned planes first; AR1 launches while halo conv1 runs
  - rsqrt via int bit-trick + 2 Newton steps on DVE: the scalar engine
    keeps the Gelu table loaded -> no ACT_TABLE_LOAD on critical path
  - halo masks folded into gelu scale/bias (gelu(0*x+0) == 0)
  - gelu h1 in half-planes ordered to unblock conv2 bank 0 early
  - gelu h2 one-shot [128,8192] with accum_out = SE partial mean
"""

import sys
sys.path.insert(0, '/opt/trn_rl_repo')

import numpy as np
import ml_dtypes

import concourse.bass as bass
import concourse.bacc as bacc
import concourse.tile as tile
import concourse.mybir as mybir
from concourse.bass_utils import run_bass_kernel_spmd

F32 = mybir.dt.float32
I32 = mybir.dt.int32
BF16 = mybir.dt.bfloat16
AF = mybir.ActivationFunctionType
ALU = mybir.AluOpType

N_CORES = 8
S = 16
CIN = 32
HID = 128
EPS = 1e-5
PLANE = S * S * S            # 4096 positions per x-plane
PPAD = 18 * 18 * 18          # padded plane (z/y/w pad 1)
NPL = 4                      # stored planes per core (2 owned + 2 halo)
POS = 2 * PLANE              # owned positions per core
P_SP = S ** 4                # 65536 global spatial positions
NX = CIN * P_SP
N1 = HID * P_SP
N3 = CIN * P_SP

_cache = {}


def _col(t, i):
    return t[:, i:i + 1]


def build_program(trace_scopes=False):
    nc = bacc.Bacc("TRN2", target_bir_lowering=False, debug=False,
                   enable_asserts=False, num_devices=N_CORES)

    xs_d = nc.dram_tensor("xs", [128, PLANE], BF16, kind="ExternalInput").ap()
    w1_d = nc.dram_tensor("w1rep", [128, 128], BF16, kind="ExternalInput").ap()
    w2_d = nc.dram_tensor("w2t", [128, 81 * 128], BF16, kind="ExternalInput").ap()
    pp_d = nc.dram_tensor("params", [128, 192], F32, kind="ExternalInput").ap()
    id_d = nc.dram_tensor("ident", [128, 128], F32, kind="ExternalInput").ap()
    out_d = nc.dram_tensor("out", [CIN, POS], F32, kind="ExternalOutput").ap()

    with tile.TileContext(nc) as tc:
        with tc.tile_pool(name="big", bufs=1) as big, \
             tc.tile_pool(name="small", bufs=1) as small, \
             tc.tile_pool(name="scr", bufs=48) as scr, \
             tc.tile_pool(name="ps", bufs=8, space="PSUM") as ps, \
             tc.tile_pool(name="dram", bufs=1, space="DRAM") as dram:

            def stile(shape, name, pool=None, dtype=F32):
                return (pool or small).tile(shape, dtype, name=name)

            def sc(name, dtype=F32):
                return scr.tile([128, 1], dtype, tag="scr", name=name)

            # ---- persistent SBUF tensors ----
            x_sb = big.tile([128, PLANE], BF16, name="x_sb")
            w1_sb = big.tile([128, 128], BF16, name="w1_sb")
            w2_sb = big.tile([128, 81 * 128], BF16, name="w2_sb")
            pp = big.tile([128, 192], F32, name="pp")
            h1 = big.tile([128, NPL * PPAD], BF16, name="h1", tag="h1slot")
            h2 = big.tile([128, 2 * PLANE], F32, name="h2")
            h2b = big.tile([128, 2 * PLANE], BF16, name="h2b")
            # aprime (conv1 staging) later reused for the final f32 output
            aprime = big.tile([128, NPL * PLANE], BF16, name="aprime",
                              tag="apslot")

            d1i = dram.tile([8], F32, name="d1i")
            d2i = dram.tile([8], F32, name="d2i")
            d4i = dram.tile([8], F32, name="d4i")
            zrow = small.tile([1, 8], F32, name="zrow")
            nc.vector.memset(zrow, 0.0)

            # weights first (conv1's first matmul needs w1 + x chunk 0),
            # then x owned planes, then the rest
            nc.sync.dma_start(out=w1_sb, in_=w1_d)
            for sj in range(2):
                nc.sync.dma_start(out=x_sb[32 * sj:32 * sj + 32, :],
                                  in_=xs_d[32 * sj:32 * sj + 32, :])
            nc.sync.dma_start(out=pp, in_=pp_d)
            for sj in range(2, NPL):
                nc.sync.dma_start(out=x_sb[32 * sj:32 * sj + 32, :],
                                  in_=xs_d[32 * sj:32 * sj + 32, :])
            nc.sync.dma_start(out=w2_sb, in_=w2_d)
            id_sb = big.tile([128, 128], F32, name="id_sb")
            nc.sync.dma_start(out=id_sb, in_=id_d)
            nc.sync.dma_start(out=d1i, in_=zrow)
            nc.sync.dma_start(out=d2i, in_=zrow)
            nc.sync.dma_start(out=d4i, in_=zrow)

            # The ACT engine holds ONE function set at a time (walrus
            # tracks residency statically in schedule order) and Copy
            # needs no table. One early Gelu dummy loads the set during
            # the DMA wait; every real Gelu then runs load-free, and the
            # only on-path load left is the SE Sigmoid (unavoidable: Gelu
            # is needed immediately before it). The junk DMA keeps the
            # dummy from being dead-code-eliminated.
            dummy = stile([1, 1], "dummy")
            zb = stile([1, 1], "zb")
            nc.vector.memset(dummy, 0.0)
            nc.vector.memset(zb, 0.0)
            nc.scalar.activation(out=dummy, in_=dummy, func=AF.Gelu,
                                 bias=zb, scale=zb)
            djunk = dram.tile([1], F32, name="djunk")
            nc.sync.dma_start(out=djunk, in_=dummy)

            h1f5 = h1.rearrange("p (j y z w) -> p j y z w", j=NPL, y=18, z=18, w=18)
            h1pl = h1.rearrange("p (j r) -> p j r", j=NPL, r=PPAD)
            # zero h1 (padding must be 0); gelu-consumption order is local
            # planes 0(hL),1,2,3(hR): gpsimd zeroes 0,1; vector zeroes 2,3
            # after its stats work
            nc.gpsimd.memset(h1pl[:, 0, :], 0.0)
            nc.gpsimd.memset(h1pl[:, 1, :], 0.0)

            def interior(j):
                return h1f5[:, j, 1:17, 1:17, 1:17]

            ones = stile([128, 1], "ones")
            nc.vector.memset(ones, 1.0)
            # row-of-ones and a scalar 1 for PE broadcast/transpose of the
            # AllReduce results: a [128,1]<->[128] DMA is partition-strided
            # (128 scattered 4B descriptors, ~3-10us); a [1,N] row is one
            # burst, and the PE outer-product rebuilds the broadcast.
            ones_row = stile([1, 128], "ones_row")
            nc.vector.memset(ones_row, 1.0)
            one_t = stile([1, 1], "one_t")
            nc.vector.memset(one_t, 1.0)

            def bcast_readback(dsrc, n, tag):
                """DRAM row [n] -> [128, n] via row DMA + PE outer product.
                Returns the PSUM tile directly — DVE reads PSUM at full
                speed, so the GN chains skip a copy + dependency hop."""
                grow = stile([1, 8], f"grow_{tag}")
                nc.sync.dma_start(out=grow[:, 0:n], in_=dsrc[0:n])
                ps_b = ps.tile([128, 8], F32, tag="ps", name=f"psb_{tag}")
                nc.tensor.matmul(ps_b[:, 0:n], ones_row, grow[:, 0:n],
                                 start=True, stop=True)
                return ps_b

            # ---- DVE rsqrt: y = 1/sqrt(v) via bit trick + 2 Newton steps.
            # Keeps the scalar engine's Gelu table resident (no Sqrt table).
            def rsqrt_dve(out, v, tag):
                tb = sc(f"rs_i_{tag}", I32)
                vb = v.bitcast(I32)
                nc.vector.tensor_scalar(out=tb, in0=vb, scalar1=1,
                                        scalar2=None,
                                        op0=ALU.logical_shift_right)
                # magic - (v>>1), via subtract then negate (the fused
                # xor+add int form crashes the walrus backend)
                nc.vector.tensor_scalar(out=tb, in0=tb, scalar1=0x5f3759df,
                                        scalar2=None, op0=ALU.subtract)
                nc.vector.tensor_scalar(out=tb, in0=tb, scalar1=-1,
                                        scalar2=None, op0=ALU.mult)
                y = tb.bitcast(F32)
                h = sc(f"rs_h_{tag}")
                nc.vector.tensor_scalar_mul(out=h, in0=v, scalar1=0.5)
                t2 = sc(f"rs_t_{tag}")
                niter = 1
                for it in range(niter):
                    dst = out if it == niter - 1 else y
                    nc.vector.tensor_mul(t2, y, y)
                    nc.vector.tensor_mul(t2, t2, h)
                    nc.vector.tensor_scalar(out=t2, in0=t2, scalar1=-1.0,
                                            scalar2=1.5, op0=ALU.mult,
                                            op1=ALU.add)
                    nc.vector.tensor_mul(dst, y, t2)

            # ---- conv1 (bf16): A' = (W1*g0w) . x ----
            # Stored plane order [owned0, owned1, haloL, haloR]; LOC maps
            # stored idx -> local x position in padded h1. Owned planes run
            # first so GN stats + AR1 launch while halo conv1 still runs.
            LOC = (1, 2, 0, 3)
            ap5 = aprime.rearrange("p (s y z w) -> p s y z w",
                                   s=NPL, y=16, z=16, w=16)
            sta = stile([128, 16, 6], "sta")

            def conv1_plane(sj, with_stats):
                for n in range(8):
                    pt = ps.tile([128, 512], F32, tag="ps", name=f"c1_{sj}_{n}")
                    nc.tensor.matmul(
                        pt,
                        w1_sb[32 * sj:32 * sj + 32, :],
                        x_sb[32 * sj:32 * sj + 32, bass.ts(n, 512)],
                        start=True, stop=True, tile_position=(32 * sj, 0))
                    blk = bass.ts(sj * 8 + n, 512)
                    nc.scalar.copy(out=aprime[:, blk], in_=pt)
                    if with_stats:
                        nc.vector.bn_stats(out=sta[:, sj * 8 + n, :],
                                           in_=aprime[:, blk])

            conv1_plane(0, True)
            conv1_plane(1, True)

            # ---- x stats (owned planes = partitions 0:64, bf16 input) ----
            stx = stile([128, 8, 6], "stx")
            for c in range(8):
                nc.vector.bn_stats(out=stx[0:64, c, :],
                                   in_=x_sb[0:64, bass.ts(c, 512)])
            mvx = stile([128, 2], "mvx")
            nc.vector.bn_aggr(out=mvx[0:64, :], in_=stx[0:64])

            mva = stile([128, 2], "mva")
            nc.vector.bn_aggr(out=mva, in_=sta)

            pk = stile([128, 6], "pk")
            nc.vector.memset(pk, 0.0)
            # col0: SA_o = mean*POS ; col1: SAA_o = (var+mean^2)*POS
            nc.vector.tensor_scalar_mul(out=_col(pk, 0), in0=_col(mva, 0), scalar1=float(POS))
            t_a = sc("t_a")
            nc.vector.tensor_mul(t_a, _col(mva, 0), _col(mva, 0))
            nc.vector.tensor_add(t_a, t_a, _col(mva, 1))
            nc.vector.tensor_scalar_mul(out=_col(pk, 1), in0=t_a, scalar1=float(POS))
            nc.vector.tensor_mul(_col(pk, 2), _col(pp, 0), _col(pk, 0))   # u*SA
            nc.vector.tensor_mul(_col(pk, 3), _col(pp, 1), _col(pk, 0))   # v*SA
            nc.vector.tensor_scalar_mul(out=pk[0:64, 4:5], in0=mvx[0:64, 0:1], scalar1=float(PLANE))
            t_b = sc("t_b")
            nc.vector.tensor_mul(t_b[0:64], mvx[0:64, 0:1], mvx[0:64, 0:1])
            nc.vector.tensor_add(t_b[0:64], t_b[0:64], mvx[0:64, 1:2])
            nc.vector.tensor_scalar_mul(out=pk[0:64, 5:6], in0=t_b[0:64], scalar1=float(PLANE))

            ps_s1 = ps.tile([1, 6], F32, tag="ps", name="ps_s1")
            nc.tensor.matmul(ps_s1, ones, pk, start=True, stop=True)
            d1o = dram.tile([8], F32, name="d1o")
            row1 = stile([1, 6], "row1")
            nc.vector.tensor_copy(out=row1, in_=ps_s1)
            nc.sync.dma_start(out=d1i[0:6], in_=row1)
            nc.gpsimd.collective_compute(
                "AllReduce", mybir.AluOpType.add,
                replica_groups=[list(range(N_CORES))],
                ins=[d1i.opt()], outs=[d1o.opt()])

            # halo-plane conv1 runs during the AR1 mesh
            conv1_plane(2, False)
            conv1_plane(3, False)

            g1 = bcast_readback(d1o, 6, "g1")

            # ---- scalar chain (replicated on 128 partitions) ----
            def gn_mu_r(g, i_sum, i_ss, nval, tag):
                mu = stile([128, 1], f"mu_{tag}")
                nc.vector.tensor_scalar_mul(out=mu, in0=_col(g, i_sum), scalar1=1.0 / nval)
                ex2 = sc(f"ex2_{tag}")
                nc.vector.tensor_scalar_mul(out=ex2, in0=_col(g, i_ss), scalar1=1.0 / nval)
                musq = sc(f"msq_{tag}")
                nc.vector.tensor_mul(musq, mu, mu)
                var = sc(f"var_{tag}")
                # var+eps = (ex2 + EPS) - mu^2 in one fused op
                nc.vector.scalar_tensor_tensor(out=var, in0=ex2, scalar=EPS,
                                               in1=musq, op0=ALU.add,
                                               op1=ALU.subtract)
                r = stile([128, 1], f"r_{tag}")
                rsqrt_dve(r, var, tag)
                return mu, r

            # g1 cols: 0 SumSA, 1 SAA, 2 SumU.SA, 3 SumV.SA, 4 Sx, 5 Sxx
            mu0, r0 = gn_mu_r(g1, 4, 5, NX, "0")
            q = stile([128, 1], "q")
            nc.vector.tensor_mul(q, mu0, r0)
            scsa = sc("scsa")                       # Sum(c*SA) = col2 - q*col3
            nc.vector.tensor_mul(scsa, q, _col(g1, 3))
            nc.vector.tensor_sub(scsa, _col(g1, 2), scsa)
            # s_c / scc depend only on q: compute on gpsimd, concurrent
            # with the vector engine's mu1/v1 work
            s_c = sc("s_c")                         # Sum(c) = Su - q*Sv
            nc.gpsimd.tensor_mul(s_c, q, _col(pp, 11))
            nc.gpsimd.tensor_sub(s_c, _col(pp, 10), s_c)
            scc = sc("scc")                         # Sum(c^2)
            t_c = sc("t_c")
            nc.gpsimd.tensor_mul(t_c, q, _col(pp, 13))
            nc.gpsimd.tensor_scalar_mul(out=t_c, in0=t_c, scalar1=2.0)
            nc.gpsimd.tensor_sub(scc, _col(pp, 12), t_c)
            nc.gpsimd.tensor_mul(t_c, q, q)
            nc.gpsimd.tensor_mul(t_c, t_c, _col(pp, 14))
            nc.gpsimd.tensor_add(scc, scc, t_c)
            # mu1 = (r0*SA)/N1 + s_c*(P/N1); the s_c/scc scalings run on
            # gpsimd (SBUF-only operands) concurrent with the vector chain
            t_d = sc("t_d")
            nc.gpsimd.tensor_scalar_mul(out=t_d, in0=s_c,
                                        scalar1=float(P_SP) / N1)
            t_g1 = sc("t_g1")
            nc.gpsimd.tensor_scalar_mul(out=t_g1, in0=scc,
                                        scalar1=float(P_SP) / N1)
            mu1 = stile([128, 1], "mu1")
            nc.vector.tensor_mul(mu1, r0, _col(g1, 0))
            nc.vector.scalar_tensor_tensor(out=mu1, in0=mu1, scalar=1.0 / N1,
                                           in1=t_d, op0=ALU.mult, op1=ALU.add)
            # var1 = (r0^2*SAA + 2 r0 scsa)/N1 + P*scc/N1 - mu1^2 + EPS
            v1 = sc("v1")
            nc.vector.tensor_mul(v1, r0, r0)
            nc.vector.tensor_mul(v1, v1, _col(g1, 1))
            t_e = sc("t_e")
            nc.vector.tensor_mul(t_e, r0, scsa)
            nc.vector.scalar_tensor_tensor(out=v1, in0=t_e, scalar=2.0,
                                           in1=v1, op0=ALU.mult, op1=ALU.add)
            nc.vector.scalar_tensor_tensor(out=v1, in0=v1, scalar=1.0 / N1,
                                           in1=t_g1, op0=ALU.mult, op1=ALU.add)
            nc.vector.tensor_mul(t_e, mu1, mu1)
            nc.vector.scalar_tensor_tensor(out=v1, in0=t_e, scalar=-1.0,
                                           in1=v1, op0=ALU.mult, op1=ALU.add)
            nc.vector.tensor_scalar_add(out=v1, in0=v1, scalar1=EPS)
            r1 = stile([128, 1], "r1")
            rsqrt_dve(r1, v1, "1")
            al1 = stile([128, 1], "al1")
            nc.vector.tensor_mul(al1, r0, r1)
            nc.vector.tensor_mul(al1, al1, _col(pp, 2))
            # be1 path on gpsimd, concurrent with al1/masks on vector
            be1 = stile([128, 1], "be1")
            nc.gpsimd.tensor_mul(be1, q, _col(pp, 1))        # q*v
            nc.gpsimd.tensor_sub(be1, _col(pp, 0), be1)      # c = u - q*v
            nc.gpsimd.tensor_sub(be1, be1, mu1)              # c - mu1
            nc.gpsimd.tensor_mul(be1, be1, r1)
            nc.gpsimd.tensor_mul(be1, be1, _col(pp, 2))
            nc.gpsimd.tensor_add(be1, be1, _col(pp, 3))
            # halo-edge masks folded into gelu scale/bias: gelu(0*x+0) == 0
            al1L = stile([128, 1], "al1L")
            be1L = stile([128, 1], "be1L")
            al1R = stile([128, 1], "al1R")
            be1R = stile([128, 1], "be1R")
            nc.gpsimd.tensor_mul(al1L, al1, _col(pp, 8))
            nc.gpsimd.tensor_mul(be1L, be1, _col(pp, 8))
            nc.vector.tensor_mul(al1R, al1, _col(pp, 9))
            nc.vector.tensor_mul(be1R, be1, _col(pp, 9))

            # PE p-state keep-warm: ~2.6us of junk matmuls gated on al1 so
            # they run in the window between the chain and conv2's first
            # bank (a cooled PE runs ~2x slow for its first ~3us)
            al1b = stile([128, 1], "al1b", dtype=BF16)
            nc.vector.tensor_copy(out=al1b, in_=al1)
            jnk = ps.tile([1, 512], F32, tag="ps", name="jnk")
            for n in range(12):
                nc.tensor.matmul(jnk, al1b, x_sb[:, bass.ts(n % 8, 512)],
                                 start=True, stop=True)

            # vector finishes the remaining h1 plane zeroing
            nc.vector.memset(h1pl[:, 2, :], 0.0)
            nc.vector.memset(h1pl[:, 3, :], 0.0)

            # ---- h1 = gelu(alpha1*A' + beta1) in quarter-planes ----
            # local plane order (0=haloL,1,2,3=haloR); conv2 bank b needs
            # y rows [2b, 2b+4) of local planes 0..2, so after the first
            # three quarter-gelus (~3us) bank 0 can start.
            SB = {0: (al1L, be1L), 1: (al1, be1), 2: (al1, be1),
                  3: (al1R, be1R)}
            quarters = [(lj, qq) for qq in range(4) for lj in range(3)]
            quarters += [(3, qq) for qq in range(4)]
            INV = (2, 0, 1, 3)   # local plane -> stored plane
            for (lj, qq) in quarters:
                sj = INV[lj]
                alx, bex = SB[lj]
                nc.scalar.activation(
                    out=h1f5[:, lj, 1 + 4 * qq:5 + 4 * qq, 1:17, 1:17],
                    in_=ap5[:, sj, 4 * qq:4 * qq + 4],
                    func=AF.Gelu, bias=bex, scale=alx)

            # ---- conv2: 3^4, 81 taps, accumulate in PSUM ----
            h1r5 = h1f5
            w2r = w2_sb
            sth = stile([128, 16, 6], "sth")
            for j in range(2):
                for b in range(8):
                    pt = ps.tile([128, 512], F32, tag="ps", name=f"c2_{j}_{b}")
                    t = 0
                    for dx in range(3):
                        for dy in range(3):
                            for dz in range(3):
                                for dw in range(3):
                                    mov = h1r5[:, j + dx,
                                               2 * b + dy:2 * b + dy + 2,
                                               dz:dz + 16, dw:dw + 16]
                                    nc.tensor.matmul(pt, w2r[:, bass.ts(t, 128)],
                                                     mov,
                                                     start=(t == 0), stop=(t == 80))
                                    t += 1
                    blk = bass.ts(j * 8 + b, 512)
                    # stats straight from PSUM so they don't serialize
                    # behind the eviction (matters for the last bank,
                    # which gates AR2)
                    nc.vector.bn_stats(out=sth[:, j * 8 + b, :], in_=pt)
                    nc.scalar.copy(out=h2[:, blk], in_=pt)

            mvh = stile([128, 2], "mvh")
            nc.vector.bn_aggr(out=mvh, in_=sth)
            pk2 = stile([128, 2], "pk2")
            nc.vector.tensor_scalar_mul(out=_col(pk2, 0), in0=_col(mvh, 0), scalar1=float(POS))
            t_f = sc("t_f")
            nc.vector.tensor_mul(t_f, _col(mvh, 0), _col(mvh, 0))
            nc.vector.tensor_add(t_f, t_f, _col(mvh, 1))
            nc.vector.tensor_scalar_mul(out=_col(pk2, 1), in0=t_f, scalar1=float(POS))
            ps_s2 = ps.tile([1, 2], F32, tag="ps", name="ps_s2")
            nc.tensor.matmul(ps_s2, ones, pk2, start=True, stop=True)
            d2o = dram.tile([8], F32, name="d2o")
            row2 = stile([1, 2], "row2")
            nc.vector.tensor_copy(out=row2, in_=ps_s2)
            nc.sync.dma_start(out=d2i[0:2], in_=row2)
            nc.gpsimd.collective_compute(
                "AllReduce", mybir.AluOpType.add,
                replica_groups=[list(range(N_CORES))],
                ins=[d2i.opt()], outs=[d2o.opt()])
            g2 = bcast_readback(d2o, 2, "g2")

            mu2, r2 = gn_mu_r(g2, 0, 1, N1, "2")
            al2 = stile([128, 1], "al2")
            nc.vector.tensor_mul(al2, r2, _col(pp, 4))
            be2 = stile([128, 1], "be2")
            nc.vector.tensor_mul(be2, mu2, al2)
            nc.vector.tensor_sub(be2, _col(pp, 5), be2)

            # ---- gelu(GN2) one-shot; accum_out is the SE partial sum ----
            m_col = stile([128, 1], "m_col")
            nc.scalar.activation(out=h2b, in_=h2,
                                 func=AF.Gelu, bias=be2, scale=al2,
                                 accum_out=m_col)
            # transpose [128,1] -> [1,128] via identity matmul: a
            # partition-strided SBUF->DRAM DMA does 128 scattered 4B reads
            # (~10us!) and stalls the AR3 trigger; a [1,128] row is one
            # contiguous burst.
            ps_t = ps.tile([1, 128], F32, tag="ps", name="ps_t")
            nc.tensor.matmul(ps_t, m_col, id_sb, start=True, stop=True)
            m_row = stile([1, 128], "m_row")
            nc.vector.tensor_copy(out=m_row, in_=ps_t)
            d3i = dram.tile([128], F32, name="d3i")
            d3o = dram.tile([128], F32, name="d3o")
            nc.sync.dma_start(out=d3i, in_=m_row)
            nc.gpsimd.collective_compute(
                "AllReduce", mybir.AluOpType.add,
                replica_groups=[list(range(N_CORES))],
                ins=[d3i.opt()], outs=[d3o.opt()])
            # read the 128-float result as one row, transpose back to a
            # column via PE (rhs = [1,1] one)
            m_row2 = stile([1, 128], "m_row2")
            nc.sync.dma_start(out=m_row2, in_=d3o)
            ps_mt = ps.tile([128, 1], F32, tag="ps", name="ps_mt")
            nc.tensor.matmul(ps_mt, m_row2, one_t, start=True, stop=True)
            m_sb = ps_mt

            # ---- SE MLP (tiny, replicated on every core) ----
            m_mean = stile([128, 1], "m_mean")
            nc.vector.tensor_scalar_mul(out=m_mean, in0=m_sb, scalar1=1.0 / P_SP)
            ps_se1 = ps.tile([8, 1], F32, tag="ps", name="ps_se1")
            nc.tensor.matmul(ps_se1, pp[:, 16:24], m_mean, start=True, stop=True)
            y1g = stile([8, 1], "y1g")
            nc.scalar.activation(out=y1g, in_=ps_se1, func=AF.Gelu)
            ps_se2 = ps.tile([128, 1], F32, tag="ps", name="ps_se2")
            nc.tensor.matmul(ps_se2, pp[0:8, 56:184], y1g, start=True, stop=True)
            s_sb = stile([128, 1], "s_sb")
            nc.scalar.activation(out=s_sb, in_=ps_se2, func=AF.Sigmoid)
            w3s = small.tile([128, 32], BF16, name="w3s")
            nc.vector.tensor_scalar_mul(out=w3s, in0=pp[:, 24:56], scalar1=s_sb)

            # ---- conv3 (bf16), 3-up packed: blocks n=3g+j land on
            # partition band 32j of PSUM group g -> 6 evictions instead of
            # 16. Band j>=1 of group 5 is zero-padded so the uniform
            # 3072-sample stats stay exact (zeros don't change sums).
            y3p = big.tile([96, 6 * 512], BF16, name="y3p", tag="h1slot")
            nc.vector.memset(y3p[32:64, 5 * 512:6 * 512], 0.0)
            nc.vector.memset(y3p[64:96, 5 * 512:6 * 512], 0.0)
            st3 = stile([96, 6, 6], "st3")
            for g in range(6):
                nj = 3 if g < 5 else 1
                pt3 = ps.tile([96, 512], F32, tag="ps", name=f"c3_{g}")
                for j in range(nj):
                    n = 3 * g + j
                    nc.tensor.matmul(pt3[32 * j:32 * j + 32, :], w3s,
                                     h2b[:, bass.ts(n, 512)],
                                     start=True, stop=True)
                blk = bass.ts(g, 512)
                nc.scalar.copy(out=y3p[0:32 * nj, blk], in_=pt3[0:32 * nj, :])
                nc.vector.bn_stats(out=st3[:, g, :], in_=y3p[0:96, blk])
            mv3 = stile([96, 2], "mv3")
            nc.vector.bn_aggr(out=mv3, in_=st3)
            pk3 = stile([128, 2], "pk3")
            nc.vector.memset(pk3, 0.0)
            NS3 = 6.0 * 512.0
            nc.vector.tensor_scalar_mul(out=pk3[0:96, 0:1], in0=mv3[:, 0:1], scalar1=NS3)
            t_g = sc("t_g")
            nc.vector.tensor_mul(t_g[0:96], mv3[:, 0:1], mv3[:, 0:1])
            nc.vector.tensor_add(t_g[0:96], t_g[0:96], mv3[:, 1:2])
            nc.vector.tensor_scalar_mul(out=pk3[0:96, 1:2], in0=t_g[0:96], scalar1=NS3)
            ps_s3 = ps.tile([1, 2], F32, tag="ps", name="ps_s3")
            nc.tensor.matmul(ps_s3, ones, pk3, start=True, stop=True)
            d4o = dram.tile([8], F32, name="d4o")
            row3 = stile([1, 2], "row3")
            nc.vector.tensor_copy(out=row3, in_=ps_s3)
            nc.sync.dma_start(out=d4i[0:2], in_=row3)
            nc.gpsimd.collective_compute(
                "AllReduce", mybir.AluOpType.add,
                replica_groups=[list(range(N_CORES))],
                ins=[d4i.opt()], outs=[d4o.opt()])
            g4 = bcast_readback(d4o, 2, "g4")

            mu3, r3 = gn_mu_r(g4, 0, 1, N3, "3")
            al3 = stile([128, 1], "al3")
            nc.vector.tensor_mul(al3, r3, _col(pp, 6))
            be3 = stile([128, 1], "be3")
            nc.vector.tensor_mul(be3, mu3, al3)
            nc.vector.tensor_sub(be3, _col(pp, 7), be3)

            # final affine on the packed layout (one DVE op), then three
            # band-unpack DMAs rebuild [32, POS] in DRAM. The f32 result
            # reuses aprime's SBUF slot. pp cols 6/7 hold gn3 w/b
            # replicated per 32-partition band.
            y3f = big.tile([96, 6 * 512], F32, name="y3f", tag="apslot")
            nc.vector.tensor_scalar(out=y3f, in0=y3p[0:96, :],
                                    scalar1=al3[0:96], scalar2=be3[0:96],
                                    op0=mybir.AluOpType.mult,
                                    op1=mybir.AluOpType.add)
            for j in range(3):
                ng = 6 if j == 0 else 5
                sb_ap = y3f[32 * j:32 * j + 32].rearrange(
                    "p (g i) -> p g i", g=6, i=512)[:, 0:ng, :]
                dram_ap = bass.AP(tensor=out_d.tensor,
                                  offset=out_d.offset + 512 * j,
                                  ap=[[POS, 32], [3 * 512, ng], [1, 512]])
                nc.sync.dma_start(out=dram_ap, in_=sb_ap)

    nc.compile()
    return nc


def _host_prep(inputs):
    x = np.asarray(inputs['x'], np.float32).reshape(CIN, S, S, S, S)
    g0w = np.asarray(inputs['g0_w'], np.float32)
    g0b = np.asarray(inputs['g0_b'], np.float32)
    W1 = np.asarray(inputs['w1'], np.float32).reshape(HID, CIN)
    gn1w = np.asarray(inputs['gn1_w'], np.float32)
    gn1b = np.asarray(inputs['gn1_b'], np.float32)
    w2 = np.asarray(inputs['w2'], np.float32).reshape(HID, HID, 3, 3, 3, 3)
    gn2w = np.asarray(inputs['gn2_w'], np.float32)
    gn2b = np.asarray(inputs['gn2_b'], np.float32)
    se1 = np.asarray(inputs['se_w1'], np.float32)   # [8,128]
    se2 = np.asarray(inputs['se_w2'], np.float32)   # [128,8]
    W3 = np.asarray(inputs['w3'], np.float32).reshape(CIN, HID)
    gn3w = np.asarray(inputs['gn3_w'], np.float32)
    gn3b = np.asarray(inputs['gn3_b'], np.float32)

    w1fold = W1 * g0w[None, :]
    w1rep = np.zeros((128, 128), np.float32)
    for j in range(4):
        w1rep[32 * j:32 * j + 32, :] = w1fold.T
    w1rep = w1rep.astype(ml_dtypes.bfloat16)
    u = W1 @ g0b
    v = W1 @ g0w
    w2t = np.ascontiguousarray(
        w2.transpose(1, 2, 3, 4, 5, 0).reshape(HID, 81 * HID)).astype(
            ml_dtypes.bfloat16)

    params = np.zeros((128, 192), np.float32)
    params[:, 0] = u
    params[:, 1] = v
    params[:, 2] = gn1w
    params[:, 3] = gn1b
    params[:, 4] = gn2w
    params[:, 5] = gn2b
    params[0:96, 6] = np.tile(gn3w, 3)
    params[0:96, 7] = np.tile(gn3b, 3)
    params[:, 10] = u.sum()
    params[:, 11] = v.sum()
    params[:, 12] = (u * u).sum()
    params[:, 13] = (u * v).sum()
    params[:, 14] = (v * v).sum()
    params[:, 16:24] = se1.T
    params[:, 24:56] = W3.T
    params[0:8, 56:184] = se2.T

    xp = np.zeros((CIN, S + 2, S, S, S), np.float32)
    xp[:, 1:S + 1] = x

    in_maps = []
    for k in range(N_CORES):
        p = params.copy()
        p[:, 8] = 0.0 if k == 0 else 1.0
        p[:, 9] = 0.0 if k == N_CORES - 1 else 1.0
        # stored plane order: [owned0, owned1, haloL, haloR]
        idx = [2 * k + 1, 2 * k + 2, 2 * k, 2 * k + 3]
        shard = np.ascontiguousarray(
            xp[:, idx].transpose(1, 0, 2, 3, 4).reshape(128, PLANE)).astype(
                ml_dtypes.bfloat16)
        in_maps.append({"xs": shard, "w1rep": w1rep, "w2t": w2t, "params": p,
                        "ident": np.eye(128, dtype=np.float32)})
    return in_maps


def kernel(**inputs):
    if "nc" not in _cache:
        _cache["nc"] = build_program()
    nc = _cache["nc"]
    in_maps = _host_prep(inputs)
    res = run_bass_kernel_spmd(nc, in_maps, core_ids=list(range(N_CORES)))
    out = np.empty((1, CIN, S, S, S, S), np.float32)
    for k in range(N_CORES):
        out[0, :, 2 * k:2 * k + 2] = res.results[k]["out"].reshape(CIN, 2, S, S, S)
    return out


def run_traced(inputs):
    """Like kernel() but with NTFF tracing; returns (out, BassKernelResults)."""
    if "nc" not in _cache:
        _cache["nc"] = build_program()
    nc = _cache["nc"]
    in_maps = _host_prep(inputs)
    res = run_bass_kernel_spmd(nc, in_maps, core_ids=list(range(N_CORES)),
                               trace=True)
    out = np.empty((1, CIN, S, S, S, S), np.float32)
    for k in range(N_CORES):
        out[0, :, 2 * k:2 * k + 2] = res.results[k]["out"].reshape(CIN, 2, S, S, S)
    return out, res



# revision 23
# speedup vs baseline: 1.0181x; 1.0181x over previous
"""MBConv v2 baseline (restored) — see kernel_v3.py for the WIP variant."""

import sys
sys.path.insert(0, '/opt/trn_rl_repo')

import numpy as np
import ml_dtypes

import concourse.bass as bass
import concourse.bacc as bacc
import concourse.tile as tile
import concourse.mybir as mybir
from concourse.bass_utils import run_bass_kernel_spmd

F32 = mybir.dt.float32
I32 = mybir.dt.int32
BF16 = mybir.dt.bfloat16
AF = mybir.ActivationFunctionType
ALU = mybir.AluOpType

N_CORES = 8
S = 16
CIN = 32
HID = 128
EPS = 1e-5
PLANE = S * S * S
PPAD = 18 * 18 * 18
NPL = 4
POS = 2 * PLANE
P_SP = S ** 4
NX = CIN * P_SP
N1 = HID * P_SP
N3 = CIN * P_SP

_cache = {}


def _col(t, i):
    return t[:, i:i + 1]


def build_program(trace_scopes=False):
    nc = bacc.Bacc("TRN2", target_bir_lowering=False, debug=False,
                   enable_asserts=False, num_devices=N_CORES)

    xs_d = nc.dram_tensor("xs", [128, PLANE], BF16, kind="ExternalInput").ap()
    w1_d = nc.dram_tensor("w1rep", [128, 128], BF16, kind="ExternalInput").ap()
    w2_d = nc.dram_tensor("w2t", [128, 81 * 128], BF16, kind="ExternalInput").ap()
    pp_d = nc.dram_tensor("params", [128, 192], F32, kind="ExternalInput").ap()
    id_d = nc.dram_tensor("ident", [128, 128], F32, kind="ExternalInput").ap()
    out_d = nc.dram_tensor("out", [CIN, POS], F32, kind="ExternalOutput").ap()

    with tile.TileContext(nc) as tc:
        with tc.tile_pool(name="big", bufs=1) as big, \
             tc.tile_pool(name="small", bufs=1) as small, \
             tc.tile_pool(name="scr", bufs=48) as scr, \
             tc.tile_pool(name="ps", bufs=8, space="PSUM") as ps, \
             tc.tile_pool(name="dram", bufs=1, space="DRAM") as dram:

            def stile(shape, name, pool=None, dtype=F32):
                return (pool or small).tile(shape, dtype, name=name)

            def sc(name, dtype=F32):
                return scr.tile([128, 1], dtype, tag="scr", name=name)

            x_sb = big.tile([128, PLANE], BF16, name="x_sb")
            w1_sb = big.tile([128, 128], BF16, name="w1_sb")
            w2_sb = big.tile([128, 81 * 128], BF16, name="w2_sb")
            pp = big.tile([128, 192], F32, name="pp")
            h1 = big.tile([128, NPL * PPAD], BF16, name="h1", tag="h1slot")
            h2 = big.tile([128, 2 * PLANE], BF16, name="h2")
            h2b = big.tile([128, 2 * PLANE], BF16, name="h2b")
            aprime = big.tile([128, NPL * PLANE], BF16, name="aprime",
                              tag="apslot")

            d1i = dram.tile([8], F32, name="d1i")
            d2i = dram.tile([8], F32, name="d2i")
            d4i = dram.tile([8], F32, name="d4i")
            wu_i = dram.tile([8], F32, name="wu_i")
            wu_o = dram.tile([8], F32, name="wu_o")
            zrow = small.tile([1, 8], F32, name="zrow")
            nc.vector.memset(zrow, 0.0)
            nc.sync.dma_start(out=wu_i, in_=zrow)
            nc.gpsimd.collective_compute(
                "AllReduce", mybir.AluOpType.add,
                replica_groups=[list(range(N_CORES))],
                ins=[wu_i.opt()], outs=[wu_o.opt()])

            nc.sync.dma_start(out=w1_sb, in_=w1_d)
            for sj in range(2):
                nc.sync.dma_start(out=x_sb[32 * sj:32 * sj + 32, :],
                                  in_=xs_d[32 * sj:32 * sj + 32, :])
            nc.sync.dma_start(out=pp, in_=pp_d)
            for sj in range(2, NPL):
                nc.sync.dma_start(out=x_sb[32 * sj:32 * sj + 32, :],
                                  in_=xs_d[32 * sj:32 * sj + 32, :])
            nc.sync.dma_start(out=w2_sb, in_=w2_d)
            id_sb = big.tile([128, 128], F32, name="id_sb")
            nc.sync.dma_start(out=id_sb, in_=id_d)
            nc.sync.dma_start(out=d1i, in_=zrow)
            nc.sync.dma_start(out=d2i, in_=zrow)
            nc.sync.dma_start(out=d4i, in_=zrow)

            dummy = stile([1, 1], "dummy")
            zb = stile([1, 1], "zb")
            nc.vector.memset(dummy, 0.0)
            nc.vector.memset(zb, 0.0)
            nc.scalar.activation(out=dummy, in_=dummy, func=AF.Gelu,
                                 bias=zb, scale=zb)
            djunk = dram.tile([1], F32, name="djunk")
            nc.sync.dma_start(out=djunk, in_=dummy)

            h1f5 = h1.rearrange("p (j y z w) -> p j y z w", j=NPL, y=18, z=18, w=18)
            for _j in range(NPL):
                nc.gpsimd.memset(h1f5[:, _j, 0, :, :], 0.0)
                nc.gpsimd.memset(h1f5[:, _j, 17, :, :], 0.0)
                nc.gpsimd.memset(h1f5[:, _j, 1:17, 0, :], 0.0)
                nc.gpsimd.memset(h1f5[:, _j, 1:17, 17, :], 0.0)
                nc.gpsimd.memset(h1f5[:, _j, 1:17, 1:17, 0], 0.0)
                nc.gpsimd.memset(h1f5[:, _j, 1:17, 1:17, 17], 0.0)

            ones = stile([128, 1], "ones")
            nc.vector.memset(ones, 1.0)
            pk = stile([128, 6], "pk")
            nc.vector.memset(pk, 0.0)
            ones_row = stile([1, 128], "ones_row")
            nc.vector.memset(ones_row, 1.0)
            one_t = stile([1, 1], "one_t")
            nc.vector.memset(one_t, 1.0)

            def bcast_readback(dsrc, n, tag):
                grow = stile([1, 8], f"grow_{tag}")
                nc.sync.dma_start(out=grow[:, 0:n], in_=dsrc[0:n])
                ps_b = ps.tile([128, 8], F32, tag="ps", name=f"psb_{tag}")
                nc.tensor.matmul(ps_b[:, 0:n], ones_row, grow[:, 0:n],
                                 start=True, stop=True)
                return ps_b

            def rsqrt_dve(out, v, tag):
                tb = sc(f"rs_i_{tag}", I32)
                vb = v.bitcast(I32)
                nc.vector.tensor_scalar(out=tb, in0=vb, scalar1=1,
                                        scalar2=None,
                                        op0=ALU.logical_shift_right)
                nc.vector.tensor_scalar(out=tb, in0=tb, scalar1=0x5f3759df,
                                        scalar2=None, op0=ALU.subtract)
                nc.vector.tensor_scalar(out=tb, in0=tb, scalar1=-1,
                                        scalar2=None, op0=ALU.mult)
                y = tb.bitcast(F32)
                h = sc(f"rs_h_{tag}")
                nc.vector.tensor_scalar_mul(out=h, in0=v, scalar1=0.5)
                t2 = sc(f"rs_t_{tag}")
                niter = 1
                for it in range(niter):
                    dst = out if it == niter - 1 else y
                    nc.vector.tensor_mul(t2, y, y)
                    nc.vector.tensor_mul(t2, t2, h)
                    nc.vector.tensor_scalar(out=t2, in0=t2, scalar1=-1.0,
                                            scalar2=1.5, op0=ALU.mult,
                                            op1=ALU.add)
                    nc.vector.tensor_mul(dst, y, t2)

            LOC = (1, 2, 0, 3)
            ap5 = aprime.rearrange("p (s y z w) -> p s y z w",
                                   s=NPL, y=16, z=16, w=16)
            sta = stile([128, 16, 6], "sta")

            def conv1_plane(sj, with_stats):
                for n in range(8):
                    pt = ps.tile([128, 512], F32, tag="ps", name=f"c1_{sj}_{n}")
                    nc.tensor.matmul(
                        pt,
                        w1_sb[32 * sj:32 * sj + 32, :],
                        x_sb[32 * sj:32 * sj + 32, bass.ts(n, 512)],
                        start=True, stop=True, tile_position=(32 * sj, 0))
                    if with_stats:
                        nc.vector.bn_stats(out=sta[:, sj * 8 + n, :], in_=pt)
                    blk = bass.ts(sj * 8 + n, 512)
                    nc.scalar.copy(out=aprime[:, blk], in_=pt)

            stx = stile([128, 8, 6], "stx")
            for c in range(8):
                nc.vector.bn_stats(out=stx[0:64, c, :],
                                   in_=x_sb[0:64, bass.ts(c, 512)])
            mvx = stile([128, 2], "mvx")
            nc.vector.bn_aggr(out=mvx[0:64, :], in_=stx[0:64])

            conv1_plane(0, True)
            conv1_plane(1, True)

            mva = stile([128, 2], "mva")
            nc.vector.bn_aggr(out=mva, in_=sta)

            nc.vector.tensor_scalar_mul(out=_col(pk, 0), in0=_col(mva, 0), scalar1=float(POS))
            t_a = sc("t_a")
            nc.vector.tensor_mul(t_a, _col(mva, 0), _col(mva, 0))
            nc.vector.tensor_add(t_a, t_a, _col(mva, 1))
            nc.vector.tensor_scalar_mul(out=_col(pk, 1), in0=t_a, scalar1=float(POS))
            nc.vector.tensor_mul(_col(pk, 2), _col(pp, 0), _col(pk, 0))
            nc.vector.tensor_mul(_col(pk, 3), _col(pp, 1), _col(pk, 0))
            nc.vector.tensor_scalar_mul(out=pk[0:64, 4:5], in0=mvx[0:64, 0:1], scalar1=float(PLANE))
            t_b = sc("t_b")
            nc.vector.tensor_mul(t_b[0:64], mvx[0:64, 0:1], mvx[0:64, 0:1])
            nc.vector.tensor_add(t_b[0:64], t_b[0:64], mvx[0:64, 1:2])
            nc.vector.tensor_scalar_mul(out=pk[0:64, 5:6], in0=t_b[0:64], scalar1=float(PLANE))

            ps_s1 = ps.tile([1, 6], F32, tag="ps", name="ps_s1")
            nc.tensor.matmul(ps_s1, ones, pk, start=True, stop=True)
            d1o = dram.tile([8], F32, name="d1o")
            row1 = stile([1, 6], "row1")
            nc.vector.tensor_copy(out=row1, in_=ps_s1)
            nc.sync.dma_start(out=d1i[0:6], in_=row1)
            nc.gpsimd.collective_compute(
                "AllReduce", mybir.AluOpType.add,
                replica_groups=[list(range(N_CORES))],
                ins=[d1i.opt()], outs=[d1o.opt()])

            conv1_plane(2, False)
            conv1_plane(3, False)

            pA = ps.tile([128, 512], F32, tag="ps", name="probeA")
            for n in range(8):
                nc.tensor.matmul(pA, x_sb[:, bass.ts(n % 4, 128)],
                                 x_sb[:, bass.ts(n, 512)],
                                 start=(n == 0), stop=(n == 7))
            pB = ps.tile([128, 512], F32, tag="ps", name="probeB")
            for n in range(8):
                nc.tensor.matmul(pB, x_sb[:, 0:128],
                                 x_sb[:, bass.ts(n, 512)],
                                 start=(n == 0), stop=(n == 7))
            pC = ps.tile([128, 512], F32, tag="ps", name="probeC")
            for n in range(8):
                nc.tensor.matmul(pC, x_sb[:, bass.ts(n % 4, 128)],
                                 ap5[:, 0, (n % 8):(n % 8) + 2, 0:16, 0:16],
                                 start=(n == 0), stop=(n == 7))

            g1 = bcast_readback(d1o, 6, "g1")

            def gn_mu_r(g, i_sum, i_ss, nval, tag):
                mu = stile([128, 1], f"mu_{tag}")
                nc.vector.tensor_scalar_mul(out=mu, in0=_col(g, i_sum), scalar1=1.0 / nval)
                ex2 = sc(f"ex2_{tag}")
                nc.vector.tensor_scalar_mul(out=ex2, in0=_col(g, i_ss), scalar1=1.0 / nval)
                musq = sc(f"msq_{tag}")
                nc.vector.tensor_mul(musq, mu, mu)
                var = sc(f"var_{tag}")
                nc.vector.scalar_tensor_tensor(out=var, in0=ex2, scalar=EPS,
                                               in1=musq, op0=ALU.add,
                                               op1=ALU.subtract)
                r = stile([128, 1], f"r_{tag}")
                rsqrt_dve(r, var, tag)
                return mu, r

            mu0, r0 = gn_mu_r(g1, 4, 5, NX, "0")
            q = stile([128, 1], "q")
            nc.vector.tensor_mul(q, mu0, r0)
            scsa = sc("scsa")
            nc.vector.tensor_mul(scsa, q, _col(g1, 3))
            nc.vector.tensor_sub(scsa, _col(g1, 2), scsa)
            s_c = sc("s_c")
            nc.gpsimd.tensor_mul(s_c, q, _col(pp, 11))
            nc.gpsimd.tensor_sub(s_c, _col(pp, 10), s_c)
            scc = sc("scc")
            t_c = sc("t_c")
            nc.gpsimd.tensor_mul(t_c, q, _col(pp, 13))
            nc.gpsimd.tensor_scalar_mul(out=t_c, in0=t_c, scalar1=2.0)
            nc.gpsimd.tensor_sub(scc, _col(pp, 12), t_c)
            nc.gpsimd.tensor_mul(t_c, q, q)
            nc.gpsimd.tensor_mul(t_c, t_c, _col(pp, 14))
            nc.gpsimd.tensor_add(scc, scc, t_c)
            t_d = sc("t_d")
            nc.gpsimd.tensor_scalar_mul(out=t_d, in0=s_c,
                                        scalar1=float(P_SP) / N1)
            t_g1 = sc("t_g1")
            nc.gpsimd.tensor_scalar_mul(out=t_g1, in0=scc,
                                        scalar1=float(P_SP) / N1)
            mu1 = stile([128, 1], "mu1")
            nc.vector.tensor_mul(mu1, r0, _col(g1, 0))
            nc.vector.scalar_tensor_tensor(out=mu1, in0=mu1, scalar=1.0 / N1,
                                           in1=t_d, op0=ALU.mult, op1=ALU.add)
            v1 = sc("v1")
            nc.vector.tensor_mul(v1, r0, r0)
            nc.vector.tensor_mul(v1, v1, _col(g1, 1))
            t_e = sc("t_e")
            nc.vector.tensor_mul(t_e, r0, scsa)
            nc.vector.scalar_tensor_tensor(out=v1, in0=t_e, scalar=2.0,
                                           in1=v1, op0=ALU.mult, op1=ALU.add)
            nc.vector.scalar_tensor_tensor(out=v1, in0=v1, scalar=1.0 / N1,
                                           in1=t_g1, op0=ALU.mult, op1=ALU.add)
            nc.vector.tensor_mul(t_e, mu1, mu1)
            nc.vector.scalar_tensor_tensor(out=v1, in0=t_e, scalar=-1.0,
                                           in1=v1, op0=ALU.mult, op1=ALU.add)
            nc.vector.tensor_scalar_add(out=v1, in0=v1, scalar1=EPS)
            r1 = stile([128, 1], "r1")
            rsqrt_dve(r1, v1, "1")
            al1 = stile([128, 1], "al1")
            nc.vector.tensor_mul(al1, r0, r1)
            nc.vector.tensor_mul(al1, al1, _col(pp, 2))
            be1 = stile([128, 1], "be1")
            nc.gpsimd.tensor_mul(be1, q, _col(pp, 1))
            nc.gpsimd.tensor_sub(be1, _col(pp, 0), be1)
            nc.gpsimd.tensor_sub(be1, be1, mu1)
            nc.gpsimd.tensor_mul(be1, be1, r1)
            nc.gpsimd.tensor_mul(be1, be1, _col(pp, 2))
            nc.gpsimd.tensor_add(be1, be1, _col(pp, 3))
            al1L = stile([128, 1], "al1L")
            be1L = stile([128, 1], "be1L")
            al1R = stile([128, 1], "al1R")
            be1R = stile([128, 1], "be1R")
            nc.gpsimd.tensor_mul(al1L, al1, _col(pp, 8))
            nc.gpsimd.tensor_mul(be1L, be1, _col(pp, 8))
            nc.vector.tensor_mul(al1R, al1, _col(pp, 9))
            nc.vector.tensor_mul(be1R, be1, _col(pp, 9))

            al1b = stile([128, 1], "al1b", dtype=BF16)
            nc.vector.tensor_copy(out=al1b, in_=al1)
            jnk = ps.tile([1, 512], F32, tag="ps", name="jnk")
            for n in range(12):
                nc.tensor.matmul(jnk, al1b, x_sb[:, bass.ts(n % 8, 512)],
                                 start=True, stop=True)


            SB = {0: (al1L, be1L), 1: (al1, be1), 2: (al1, be1),
                  3: (al1R, be1R)}
            quarters = [(lj, qq) for qq in range(4) for lj in range(3)]
            quarters += [(3, qq) for qq in range(4)]
            INV = (2, 0, 1, 3)
            for (lj, qq) in quarters:
                sj = INV[lj]
                alx, bex = SB[lj]
                nc.scalar.activation(
                    out=h1f5[:, lj, 1 + 4 * qq:5 + 4 * qq, 1:17, 1:17],
                    in_=ap5[:, sj, 4 * qq:4 * qq + 4],
                    func=AF.Gelu, bias=bex, scale=alx)

            h1r5 = h1f5
            w2r = w2_sb
            sth = stile([128, 16, 6], "sth")
            for j in range(2):
                for b in range(8):
                    pt = ps.tile([128, 512], F32, tag="ps", name=f"c2_{j}_{b}")
                    t = 0
                    for dx in range(3):
                        for dy in range(3):
                            for dz in range(3):
                                for dw in range(3):
                                    mov = h1r5[:, j + dx,
                                               2 * b + dy:2 * b + dy + 2,
                                               dz:dz + 16, dw:dw + 16]
                                    nc.tensor.matmul(pt, w2r[:, bass.ts(t, 128)],
                                                     mov,
                                                     start=(t == 0), stop=(t == 80))
                                    t += 1
                    blk = bass.ts(j * 8 + b, 512)
                    nc.vector.bn_stats(out=sth[:, j * 8 + b, :], in_=pt)
                    nc.scalar.copy(out=h2[:, blk], in_=pt)

            mvh = stile([128, 2], "mvh")
            nc.vector.bn_aggr(out=mvh, in_=sth)
            pk2 = stile([128, 2], "pk2")
            nc.vector.tensor_scalar_mul(out=_col(pk2, 0), in0=_col(mvh, 0), scalar1=float(POS))
            t_f = sc("t_f")
            nc.vector.tensor_mul(t_f, _col(mvh, 0), _col(mvh, 0))
            nc.vector.tensor_add(t_f, t_f, _col(mvh, 1))
            nc.vector.tensor_scalar_mul(out=_col(pk2, 1), in0=t_f, scalar1=float(POS))
            ps_s2 = ps.tile([1, 2], F32, tag="ps", name="ps_s2")
            nc.tensor.matmul(ps_s2, ones, pk2, start=True, stop=True)
            d2o = dram.tile([8], F32, name="d2o")
            row2 = stile([1, 2], "row2")
            nc.vector.tensor_copy(out=row2, in_=ps_s2)
            nc.sync.dma_start(out=d2i[0:2], in_=row2)
            nc.gpsimd.collective_compute(
                "AllReduce", mybir.AluOpType.add,
                replica_groups=[list(range(N_CORES))],
                ins=[d2i.opt()], outs=[d2o.opt()])
            g2 = bcast_readback(d2o, 2, "g2")

            jnk2 = ps.tile([1, 512], F32, tag="ps", name="jnk2")
            for n in range(10):
                nc.tensor.matmul(jnk2, al1b, x_sb[:, bass.ts(n % 8, 512)],
                                 start=True, stop=True)

            mu2, r2 = gn_mu_r(g2, 0, 1, N1, "2")
            al2 = stile([128, 1], "al2")
            nc.vector.tensor_mul(al2, r2, _col(pp, 4))
            be2 = stile([128, 1], "be2")
            nc.vector.tensor_mul(be2, mu2, al2)
            nc.vector.tensor_sub(be2, _col(pp, 5), be2)

            m_col = stile([128, 1], "m_col")
            nc.scalar.activation(out=h2b, in_=h2,
                                 func=AF.Gelu, bias=be2, scale=al2,
                                 accum_out=m_col)
            # preload the erf/sigmoid table during the AR3 window (the SE
            # gelu runs via erf so the whole SE block needs ONE table)
            dummy2 = stile([1, 1], "dummy2")
            nc.vector.memset(dummy2, 0.0)
            nc.scalar.activation(out=dummy2, in_=dummy2, func=AF.Erf)
            nc.sync.dma_start(out=djunk, in_=dummy2)
            ps_t = ps.tile([1, 128], F32, tag="ps", name="ps_t")
            nc.tensor.matmul(ps_t, m_col, id_sb, start=True, stop=True)
            m_row = stile([1, 128], "m_row")
            nc.vector.tensor_copy(out=m_row, in_=ps_t)
            d3i = dram.tile([128], F32, name="d3i")
            d3o = dram.tile([128], F32, name="d3o")
            nc.sync.dma_start(out=d3i, in_=m_row)
            nc.gpsimd.collective_compute(
                "AllReduce", mybir.AluOpType.add,
                replica_groups=[list(range(N_CORES))],
                ins=[d3i.opt()], outs=[d3o.opt()])
            m_row2 = stile([1, 128], "m_row2")
            nc.sync.dma_start(out=m_row2, in_=d3o)
            ps_mt = ps.tile([128, 1], F32, tag="ps", name="ps_mt")
            nc.tensor.matmul(ps_mt, m_row2, one_t, start=True, stop=True)
            m_sb = ps_mt

            m_mean = stile([128, 1], "m_mean")
            nc.vector.tensor_scalar_mul(out=m_mean, in0=m_sb, scalar1=1.0 / P_SP)
            ps_se1 = ps.tile([8, 1], F32, tag="ps", name="ps_se1")
            nc.tensor.matmul(ps_se1, pp[:, 16:24], m_mean, start=True, stop=True)
            # gelu(y) = 0.5*y*(1+erf(y/sqrt2)) -- erf is in the sigmoid table
            y1e = stile([8, 1], "y1e")
            nc.scalar.activation(out=y1e, in_=ps_se1, func=AF.Erf,
                                 scale=0.70710678)
            y1g = stile([8, 1], "y1g")
            nc.vector.tensor_mul(y1g[0:8], ps_se1, y1e[0:8])
            nc.vector.tensor_add(y1g[0:8], y1g[0:8], ps_se1)
            nc.vector.tensor_scalar_mul(out=y1g[0:8], in0=y1g[0:8], scalar1=0.5)
            ps_se2 = ps.tile([128, 1], F32, tag="ps", name="ps_se2")
            nc.tensor.matmul(ps_se2, pp[0:8, 56:184], y1g[0:8], start=True, stop=True)
            s_sb = stile([128, 1], "s_sb")
            nc.scalar.activation(out=s_sb, in_=ps_se2, func=AF.Sigmoid)
            w3s = small.tile([128, 32], BF16, name="w3s")
            nc.vector.tensor_scalar_mul(out=w3s, in0=pp[:, 24:56], scalar1=s_sb)

            y3p = big.tile([96, 6 * 512], BF16, name="y3p", tag="h1slot")
            st3 = stile([96, 6, 6], "st3")
            nc.vector.memset(st3, 0.0)
            for g in range(6):
                nj = 3 if g < 5 else 1
                pt3 = ps.tile([96, 512], F32, tag="ps", name=f"c3_{g}")
                for j in range(nj):
                    n = 3 * g + j
                    nc.tensor.matmul(pt3[32 * j:32 * j + 32, :], w3s,
                                     h2b[:, bass.ts(n, 512)],
                                     start=True, stop=True)
                nc.vector.bn_stats(out=st3[0:32 * nj, g, :],
                                   in_=pt3[0:32 * nj, :])
                blk = bass.ts(g, 512)
                nc.scalar.copy(out=y3p[0:32 * nj, blk], in_=pt3[0:32 * nj, :])
            mv3 = stile([96, 2], "mv3")
            nc.vector.bn_aggr(out=mv3, in_=st3)
            pk3 = stile([128, 2], "pk3")
            nc.vector.memset(pk3, 0.0)
            # per-partition sample counts (3072 for band 0, 2560 for bands
            # 1,2 which skip group 5) live in pp col 15
            nc.vector.tensor_mul(pk3[0:96, 0:1], mv3[:, 0:1], pp[0:96, 15:16])
            t_g = sc("t_g")
            nc.vector.tensor_mul(t_g[0:96], mv3[:, 0:1], mv3[:, 0:1])
            nc.vector.tensor_add(t_g[0:96], t_g[0:96], mv3[:, 1:2])
            nc.vector.tensor_mul(pk3[0:96, 1:2], t_g[0:96], pp[0:96, 15:16])
            ps_s3 = ps.tile([1, 2], F32, tag="ps", name="ps_s3")
            nc.tensor.matmul(ps_s3, ones, pk3, start=True, stop=True)
            d4o = dram.tile([8], F32, name="d4o")
            row3 = stile([1, 2], "row3")
            nc.vector.tensor_copy(out=row3, in_=ps_s3)
            nc.sync.dma_start(out=d4i[0:2], in_=row3)
            nc.gpsimd.collective_compute(
                "AllReduce", mybir.AluOpType.add,
                replica_groups=[list(range(N_CORES))],
                ins=[d4i.opt()], outs=[d4o.opt()])
            g4 = bcast_readback(d4o, 2, "g4")

            mu3, r3 = gn_mu_r(g4, 0, 1, N3, "3")
            al3 = stile([128, 1], "al3")
            nc.vector.tensor_mul(al3, r3, _col(pp, 6))
            be3 = stile([128, 1], "be3")
            nc.vector.tensor_mul(be3, mu3, al3)
            nc.vector.tensor_sub(be3, _col(pp, 7), be3)

            y3f = big.tile([96, 6 * 512], F32, name="y3f", tag="apslot")
            for half in range(2):
                cs = half * 3 * 512
                nc.vector.tensor_scalar(out=y3f[:, cs:cs + 3 * 512],
                                        in0=y3p[0:96, cs:cs + 3 * 512],
                                        scalar1=al3[0:96], scalar2=be3[0:96],
                                        op0=mybir.AluOpType.mult,
                                        op1=mybir.AluOpType.add)
                g_lo = half * 3
                for j in range(3):
                    ng = 3 if (half == 0 or j == 0) else 2
                    sb_ap = y3f[32 * j:32 * j + 32].rearrange(
                        "p (g i) -> p g i", g=6, i=512)[:, g_lo:g_lo + ng, :]
                    dram_ap = bass.AP(
                        tensor=out_d.tensor,
                        offset=out_d.offset + 512 * j + 3 * 512 * g_lo,
                        ap=[[POS, 32], [3 * 512, ng], [1, 512]])
                    nc.sync.dma_start(out=dram_ap, in_=sb_ap)

    nc.compile()
    return nc


def _host_prep(inputs):
    x = np.asarray(inputs['x'], np.float32).reshape(CIN, S, S, S, S)
    g0w = np.asarray(inputs['g0_w'], np.float32)
    g0b = np.asarray(inputs['g0_b'], np.float32)
    W1 = np.asarray(inputs['w1'], np.float32).reshape(HID, CIN)
    gn1w = np.asarray(inputs['gn1_w'], np.float32)
    gn1b = np.asarray(inputs['gn1_b'], np.float32)
    w2 = np.asarray(inputs['w2'], np.float32).reshape(HID, HID, 3, 3, 3, 3)
    gn2w = np.asarray(inputs['gn2_w'], np.float32)
    gn2b = np.asarray(inputs['gn2_b'], np.float32)
    se1 = np.asarray(inputs['se_w1'], np.float32)
    se2 = np.asarray(inputs['se_w2'], np.float32)
    W3 = np.asarray(inputs['w3'], np.float32).reshape(CIN, HID)
    gn3w = np.asarray(inputs['gn3_w'], np.float32)
    gn3b = np.asarray(inputs['gn3_b'], np.float32)

    w1fold = W1 * g0w[None, :]
    w1rep = np.zeros((128, 128), np.float32)
    for j in range(4):
        w1rep[32 * j:32 * j + 32, :] = w1fold.T
    w1rep = w1rep.astype(ml_dtypes.bfloat16)
    u = W1 @ g0b
    v = W1 @ g0w
    w2t = np.ascontiguousarray(
        w2.transpose(1, 2, 3, 4, 5, 0).reshape(HID, 81 * HID)).astype(
            ml_dtypes.bfloat16)

    params = np.zeros((128, 192), np.float32)
    params[:, 0] = u
    params[:, 1] = v
    params[:, 2] = gn1w
    params[:, 3] = gn1b
    params[:, 4] = gn2w
    params[:, 5] = gn2b
    params[0:96, 6] = np.tile(gn3w, 3)
    params[0:96, 7] = np.tile(gn3b, 3)
    params[:, 10] = u.sum()
    params[:, 11] = v.sum()
    params[:, 12] = (u * u).sum()
    params[:, 13] = (u * v).sum()
    params[:, 14] = (v * v).sum()
    params[0:32, 15] = 6.0 * 512.0
    params[32:96, 15] = 5.0 * 512.0
    params[:, 16:24] = se1.T
    params[:, 24:56] = W3.T
    params[0:8, 56:184] = se2.T

    xp = np.zeros((CIN, S + 2, S, S, S), np.float32)
    xp[:, 1:S + 1] = x

    in_maps = []
    for k in range(N_CORES):
        p = params.copy()
        p[:, 8] = 0.0 if k == 0 else 1.0
        p[:, 9] = 0.0 if k == N_CORES - 1 else 1.0
        idx = [2 * k + 1, 2 * k + 2, 2 * k, 2 * k + 3]
        shard = np.ascontiguousarray(
            xp[:, idx].transpose(1, 0, 2, 3, 4).reshape(128, PLANE)).astype(
                ml_dtypes.bfloat16)
        in_maps.append({"xs": shard, "w1rep": w1rep, "w2t": w2t, "params": p,
                        "ident": np.eye(128, dtype=np.float32)})
    return in_maps


def kernel(**inputs):
    if "nc" not in _cache:
        _cache["nc"] = build_program()
    nc = _cache["nc"]
    in_maps = _host_prep(inputs)
    res = run_bass_kernel_spmd(nc, in_maps, core_ids=list(range(N_CORES)))
    out = np.empty((1, CIN, S, S, S, S), np.float32)
    for k in range(N_CORES):
        out[0, :, 2 * k:2 * k + 2] = res.results[k]["out"].reshape(CIN, 2, S, S, S)
    return out


def run_traced(inputs):
    if "nc" not in _cache:
        _cache["nc"] = build_program()
    nc = _cache["nc"]
    in_maps = _host_prep(inputs)
    res = run_bass_kernel_spmd(nc, in_maps, core_ids=list(range(N_CORES)),
                               trace=True)
    out = np.empty((1, CIN, S, S, S, S), np.float32)
    for k in range(N_CORES):
        out[0, :, 2 * k:2 * k + 2] = res.results[k]["out"].reshape(CIN, 2, S, S, S)
    return out, res


# revision 29
# speedup vs baseline: 1.0228x; 1.0046x over previous
"""MBConv v2 baseline (restored) — see kernel_v3.py for the WIP variant."""

import sys
sys.path.insert(0, '/opt/trn_rl_repo')

import numpy as np
import ml_dtypes

import concourse.bass as bass
import concourse.bacc as bacc
import concourse.tile as tile
import concourse.mybir as mybir
from concourse.bass_utils import run_bass_kernel_spmd

F32 = mybir.dt.float32
I32 = mybir.dt.int32
BF16 = mybir.dt.bfloat16
AF = mybir.ActivationFunctionType
ALU = mybir.AluOpType

N_CORES = 8
S = 16
CIN = 32
HID = 128
EPS = 1e-5
PLANE = S * S * S
PPAD = 18 * 18 * 18
NPL = 4
POS = 2 * PLANE
P_SP = S ** 4
NX = CIN * P_SP
N1 = HID * P_SP
N3 = CIN * P_SP

_cache = {}


def _col(t, i):
    return t[:, i:i + 1]


def build_program(trace_scopes=False):
    nc = bacc.Bacc("TRN2", target_bir_lowering=False, debug=False,
                   enable_asserts=False, num_devices=N_CORES)

    xs_d = nc.dram_tensor("xs", [128, PLANE], BF16, kind="ExternalInput").ap()
    w1_d = nc.dram_tensor("w1rep", [128, 128], BF16, kind="ExternalInput").ap()
    w2_d = nc.dram_tensor("w2t", [128, 81 * 128], BF16, kind="ExternalInput").ap()
    pp_d = nc.dram_tensor("params", [128, 192], F32, kind="ExternalInput").ap()
    id_d = nc.dram_tensor("ident", [128, 128], F32, kind="ExternalInput").ap()
    out_d = nc.dram_tensor("out", [CIN, POS], F32, kind="ExternalOutput").ap()

    with tile.TileContext(nc) as tc:
        with tc.tile_pool(name="big", bufs=1) as big, \
             tc.tile_pool(name="small", bufs=1) as small, \
             tc.tile_pool(name="scr", bufs=48) as scr, \
             tc.tile_pool(name="ps", bufs=8, space="PSUM") as ps, \
             tc.tile_pool(name="dram", bufs=1, space="DRAM") as dram:

            def stile(shape, name, pool=None, dtype=F32):
                return (pool or small).tile(shape, dtype, name=name)

            def sc(name, dtype=F32):
                return scr.tile([128, 1], dtype, tag="scr", name=name)

            x_sb = big.tile([128, PLANE], BF16, name="x_sb")
            w1_sb = big.tile([128, 128], BF16, name="w1_sb")
            w2_sb = big.tile([128, 81 * 128], BF16, name="w2_sb")
            pp = big.tile([128, 192], F32, name="pp")
            h1 = big.tile([128, NPL * PPAD], BF16, name="h1", tag="h1slot")
            h2 = big.tile([128, 2 * PLANE], BF16, name="h2")
            h2b = big.tile([128, 2 * PLANE], BF16, name="h2b")
            aprime = big.tile([128, NPL * PLANE], BF16, name="aprime",
                              tag="apslot")

            d1i = dram.tile([8], F32, name="d1i")
            d2i = dram.tile([8], F32, name="d2i")
            d4i = dram.tile([8], F32, name="d4i")
            zrow = small.tile([1, 8], F32, name="zrow")
            nc.vector.memset(zrow, 0.0)

            nc.sync.dma_start(out=w1_sb, in_=w1_d)
            for sj in range(2):
                nc.sync.dma_start(out=x_sb[32 * sj:32 * sj + 32, :],
                                  in_=xs_d[32 * sj:32 * sj + 32, :])
            nc.sync.dma_start(out=pp, in_=pp_d)
            for sj in range(2, NPL):
                nc.sync.dma_start(out=x_sb[32 * sj:32 * sj + 32, :],
                                  in_=xs_d[32 * sj:32 * sj + 32, :])
            nc.sync.dma_start(out=w2_sb, in_=w2_d)
            id_sb = big.tile([128, 128], F32, name="id_sb")
            nc.sync.dma_start(out=id_sb, in_=id_d)
            nc.sync.dma_start(out=d1i, in_=zrow)
            nc.sync.dma_start(out=d2i, in_=zrow)
            nc.sync.dma_start(out=d4i, in_=zrow)

            dummy = stile([1, 1], "dummy")
            zb = stile([1, 1], "zb")
            nc.vector.memset(dummy, 0.0)
            nc.vector.memset(zb, 0.0)
            nc.scalar.activation(out=dummy, in_=dummy, func=AF.Gelu,
                                 bias=zb, scale=zb)
            djunk = dram.tile([1], F32, name="djunk")
            nc.sync.dma_start(out=djunk, in_=dummy)

            h1f5 = h1.rearrange("p (j y z w) -> p j y z w", j=NPL, y=18, z=18, w=18)
            for _j in range(NPL):
                nc.gpsimd.memset(h1f5[:, _j, 0, :, :], 0.0)
                nc.gpsimd.memset(h1f5[:, _j, 17, :, :], 0.0)
                nc.gpsimd.memset(h1f5[:, _j, 1:17, 0, :], 0.0)
                nc.gpsimd.memset(h1f5[:, _j, 1:17, 17, :], 0.0)
                nc.gpsimd.memset(h1f5[:, _j, 1:17, 1:17, 0], 0.0)
                nc.gpsimd.memset(h1f5[:, _j, 1:17, 1:17, 17], 0.0)

            ones = stile([128, 1], "ones")
            nc.vector.memset(ones, 1.0)
            pk = stile([128, 6], "pk")
            nc.vector.memset(pk, 0.0)
            ones_row = stile([1, 128], "ones_row")
            nc.vector.memset(ones_row, 1.0)
            one_t = stile([1, 1], "one_t")
            nc.vector.memset(one_t, 1.0)

            def bcast_readback(dsrc, n, tag):
                grow = stile([1, 8], f"grow_{tag}")
                nc.sync.dma_start(out=grow[:, 0:n], in_=dsrc[0:n])
                ps_b = ps.tile([128, 8], F32, tag="ps", name=f"psb_{tag}")
                nc.tensor.matmul(ps_b[:, 0:n], ones_row, grow[:, 0:n],
                                 start=True, stop=True)
                return ps_b

            def rsqrt_dve(out, v, tag):
                tb = sc(f"rs_i_{tag}", I32)
                vb = v.bitcast(I32)
                nc.vector.tensor_scalar(out=tb, in0=vb, scalar1=1,
                                        scalar2=None,
                                        op0=ALU.logical_shift_right)
                nc.vector.tensor_scalar(out=tb, in0=tb, scalar1=0x5f3759df,
                                        scalar2=None, op0=ALU.subtract)
                nc.vector.tensor_scalar(out=tb, in0=tb, scalar1=-1,
                                        scalar2=None, op0=ALU.mult)
                y = tb.bitcast(F32)
                h = sc(f"rs_h_{tag}")
                nc.vector.tensor_scalar_mul(out=h, in0=v, scalar1=0.5)
                t2 = sc(f"rs_t_{tag}")
                niter = 1
                for it in range(niter):
                    dst = out if it == niter - 1 else y
                    nc.vector.tensor_mul(t2, y, y)
                    nc.vector.tensor_mul(t2, t2, h)
                    nc.vector.tensor_scalar(out=t2, in0=t2, scalar1=-1.0,
                                            scalar2=1.5, op0=ALU.mult,
                                            op1=ALU.add)
                    nc.vector.tensor_mul(dst, y, t2)

            LOC = (1, 2, 0, 3)
            ap5 = aprime.rearrange("p (s y z w) -> p s y z w",
                                   s=NPL, y=16, z=16, w=16)
            sta = stile([128, 16, 6], "sta")

            def conv1_plane(sj, with_stats):
                for n in range(8):
                    pt = ps.tile([128, 512], F32, tag="ps", name=f"c1_{sj}_{n}")
                    nc.tensor.matmul(
                        pt,
                        w1_sb[32 * sj:32 * sj + 32, :],
                        x_sb[32 * sj:32 * sj + 32, bass.ts(n, 512)],
                        start=True, stop=True, tile_position=(32 * sj, 0))
                    if with_stats:
                        nc.vector.bn_stats(out=sta[:, sj * 8 + n, :], in_=pt)
                    blk = bass.ts(sj * 8 + n, 512)
                    nc.scalar.copy(out=aprime[:, blk], in_=pt)

            stx = stile([128, 8, 6], "stx")
            for c in range(8):
                nc.vector.bn_stats(out=stx[0:64, c, :],
                                   in_=x_sb[0:64, bass.ts(c, 512)])
            mvx = stile([128, 2], "mvx")
            nc.vector.bn_aggr(out=mvx[0:64, :], in_=stx[0:64])

            conv1_plane(0, True)
            conv1_plane(1, True)

            mva = stile([128, 2], "mva")
            nc.vector.bn_aggr(out=mva, in_=sta)

            nc.vector.tensor_scalar_mul(out=_col(pk, 0), in0=_col(mva, 0), scalar1=float(POS))
            t_a = sc("t_a")
            nc.vector.tensor_mul(t_a, _col(mva, 0), _col(mva, 0))
            nc.vector.tensor_add(t_a, t_a, _col(mva, 1))
            nc.vector.tensor_scalar_mul(out=_col(pk, 1), in0=t_a, scalar1=float(POS))
            nc.vector.tensor_mul(_col(pk, 2), _col(pp, 0), _col(pk, 0))
            nc.vector.tensor_mul(_col(pk, 3), _col(pp, 1), _col(pk, 0))
            nc.vector.tensor_scalar_mul(out=pk[0:64, 4:5], in0=mvx[0:64, 0:1], scalar1=float(PLANE))
            t_b = sc("t_b")
            nc.vector.tensor_mul(t_b[0:64], mvx[0:64, 0:1], mvx[0:64, 0:1])
            nc.vector.tensor_add(t_b[0:64], t_b[0:64], mvx[0:64, 1:2])
            nc.vector.tensor_scalar_mul(out=pk[0:64, 5:6], in0=t_b[0:64], scalar1=float(PLANE))

            ps_s1 = ps.tile([1, 6], F32, tag="ps", name="ps_s1")
            nc.tensor.matmul(ps_s1, ones, pk, start=True, stop=True)
            d1o = dram.tile([8], F32, name="d1o")
            row1 = stile([1, 6], "row1")
            nc.vector.tensor_copy(out=row1, in_=ps_s1)
            nc.sync.dma_start(out=d1i[0:6], in_=row1)
            nc.gpsimd.collective_compute(
                "AllReduce", mybir.AluOpType.add,
                replica_groups=[list(range(N_CORES))],
                ins=[d1i.opt()], outs=[d1o.opt()])

            conv1_plane(2, False)
            conv1_plane(3, False)

            pA = ps.tile([128, 512], F32, tag="ps", name="probeA")
            for n in range(8):
                nc.tensor.matmul(pA, x_sb[:, bass.ts(n % 4, 128)],
                                 x_sb[:, bass.ts(n, 512)],
                                 start=(n == 0), stop=(n == 7))
            pB = ps.tile([128, 512], F32, tag="ps", name="probeB")
            for n in range(8):
                nc.tensor.matmul(pB, x_sb[:, 0:128],
                                 x_sb[:, bass.ts(n, 512)],
                                 start=(n == 0), stop=(n == 7))
            pC = ps.tile([128, 512], F32, tag="ps", name="probeC")
            for n in range(8):
                nc.tensor.matmul(pC, x_sb[:, bass.ts(n % 4, 128)],
                                 ap5[:, 0, (n % 8):(n % 8) + 2, 0:16, 0:16],
                                 start=(n == 0), stop=(n == 7))

            g1 = bcast_readback(d1o, 6, "g1")

            def gn_mu_r(g, i_sum, i_ss, nval, tag):
                mu = stile([128, 1], f"mu_{tag}")
                nc.vector.tensor_scalar_mul(out=mu, in0=_col(g, i_sum), scalar1=1.0 / nval)
                ex2 = sc(f"ex2_{tag}")
                nc.vector.tensor_scalar_mul(out=ex2, in0=_col(g, i_ss), scalar1=1.0 / nval)
                musq = sc(f"msq_{tag}")
                nc.vector.tensor_mul(musq, mu, mu)
                var = sc(f"var_{tag}")
                nc.vector.scalar_tensor_tensor(out=var, in0=ex2, scalar=EPS,
                                               in1=musq, op0=ALU.add,
                                               op1=ALU.subtract)
                r = stile([128, 1], f"r_{tag}")
                rsqrt_dve(r, var, tag)
                return mu, r

            mu0, r0 = gn_mu_r(g1, 4, 5, NX, "0")
            q = stile([128, 1], "q")
            nc.vector.tensor_mul(q, mu0, r0)
            scsa = sc("scsa")
            nc.vector.tensor_mul(scsa, q, _col(g1, 3))
            nc.vector.tensor_sub(scsa, _col(g1, 2), scsa)
            s_c = sc("s_c")
            nc.gpsimd.tensor_mul(s_c, q, _col(pp, 11))
            nc.gpsimd.tensor_sub(s_c, _col(pp, 10), s_c)
            scc = sc("scc")
            t_c = sc("t_c")
            nc.gpsimd.tensor_mul(t_c, q, _col(pp, 13))
            nc.gpsimd.tensor_scalar_mul(out=t_c, in0=t_c, scalar1=2.0)
            nc.gpsimd.tensor_sub(scc, _col(pp, 12), t_c)
            nc.gpsimd.tensor_mul(t_c, q, q)
            nc.gpsimd.tensor_mul(t_c, t_c, _col(pp, 14))
            nc.gpsimd.tensor_add(scc, scc, t_c)
            t_d = sc("t_d")
            nc.gpsimd.tensor_scalar_mul(out=t_d, in0=s_c,
                                        scalar1=float(P_SP) / N1)
            t_g1 = sc("t_g1")
            nc.gpsimd.tensor_scalar_mul(out=t_g1, in0=scc,
                                        scalar1=float(P_SP) / N1)
            mu1 = stile([128, 1], "mu1")
            nc.vector.tensor_mul(mu1, r0, _col(g1, 0))
            nc.vector.scalar_tensor_tensor(out=mu1, in0=mu1, scalar=1.0 / N1,
                                           in1=t_d, op0=ALU.mult, op1=ALU.add)
            v1 = sc("v1")
            nc.vector.tensor_mul(v1, r0, r0)
            nc.vector.tensor_mul(v1, v1, _col(g1, 1))
            t_e = sc("t_e")
            nc.vector.tensor_mul(t_e, r0, scsa)
            nc.vector.scalar_tensor_tensor(out=v1, in0=t_e, scalar=2.0,
                                           in1=v1, op0=ALU.mult, op1=ALU.add)
            nc.vector.scalar_tensor_tensor(out=v1, in0=v1, scalar=1.0 / N1,
                                           in1=t_g1, op0=ALU.mult, op1=ALU.add)
            nc.vector.tensor_mul(t_e, mu1, mu1)
            nc.vector.scalar_tensor_tensor(out=v1, in0=t_e, scalar=-1.0,
                                           in1=v1, op0=ALU.mult, op1=ALU.add)
            nc.vector.tensor_scalar_add(out=v1, in0=v1, scalar1=EPS)
            r1 = stile([128, 1], "r1")
            rsqrt_dve(r1, v1, "1")
            al1 = stile([128, 1], "al1")
            nc.vector.tensor_mul(al1, r0, r1)
            nc.vector.tensor_mul(al1, al1, _col(pp, 2))
            be1 = stile([128, 1], "be1")
            nc.gpsimd.tensor_mul(be1, q, _col(pp, 1))
            nc.gpsimd.tensor_sub(be1, _col(pp, 0), be1)
            nc.gpsimd.tensor_sub(be1, be1, mu1)
            nc.gpsimd.tensor_mul(be1, be1, r1)
            nc.gpsimd.tensor_mul(be1, be1, _col(pp, 2))
            nc.gpsimd.tensor_add(be1, be1, _col(pp, 3))
            al1L = stile([128, 1], "al1L")
            be1L = stile([128, 1], "be1L")
            al1R = stile([128, 1], "al1R")
            be1R = stile([128, 1], "be1R")
            nc.gpsimd.tensor_mul(al1L, al1, _col(pp, 8))
            nc.gpsimd.tensor_mul(be1L, be1, _col(pp, 8))
            nc.vector.tensor_mul(al1R, al1, _col(pp, 9))
            nc.vector.tensor_mul(be1R, be1, _col(pp, 9))

            al1b = stile([128, 1], "al1b", dtype=BF16)
            nc.vector.tensor_copy(out=al1b, in_=al1)
            jnk = ps.tile([1, 512], F32, tag="ps", name="jnk")
            for n in range(12):
                nc.tensor.matmul(jnk, al1b, x_sb[:, bass.ts(n % 8, 512)],
                                 start=True, stop=True)


            SB = {0: (al1L, be1L), 1: (al1, be1), 2: (al1, be1),
                  3: (al1R, be1R)}
            quarters = [(lj, qq) for lj in range(4) for qq in range(4)]
            INV = (2, 0, 1, 3)
            for (lj, qq) in quarters:
                sj = INV[lj]
                alx, bex = SB[lj]
                nc.scalar.activation(
                    out=h1f5[:, lj, 1 + 4 * qq:5 + 4 * qq, 1:17, 1:17],
                    in_=ap5[:, sj, 4 * qq:4 * qq + 4],
                    func=AF.Gelu, bias=bex, scale=alx)

            h1r5 = h1f5
            w2r = w2_sb
            sth = stile([128, 16, 6], "sth")
            for j in range(2):
                pts = [ps.tile([128, 512], F32, tag="ps", name=f"c2_{j}_{b}")
                       for b in range(8)]
                t = 0
                for dx in range(3):
                    for dy in range(3):
                        for dz in range(3):
                            for dw in range(3):
                                wcol = w2r[:, bass.ts(t, 128)]
                                for b in range(8):
                                    mov = h1r5[:, j + dx,
                                               2 * b + dy:2 * b + dy + 2,
                                               dz:dz + 16, dw:dw + 16]
                                    nc.tensor.matmul(pts[b], wcol, mov,
                                                     start=(t == 0), stop=(t == 80))
                                t += 1
                for b in range(8):
                    blk = bass.ts(j * 8 + b, 512)
                    nc.vector.bn_stats(out=sth[:, j * 8 + b, :], in_=pts[b])
                    nc.scalar.copy(out=h2[:, blk], in_=pts[b])

            mvh = stile([128, 2], "mvh")
            nc.vector.bn_aggr(out=mvh, in_=sth)
            pk2 = stile([128, 2], "pk2")
            nc.vector.tensor_scalar_mul(out=_col(pk2, 0), in0=_col(mvh, 0), scalar1=float(POS))
            t_f = sc("t_f")
            nc.vector.tensor_mul(t_f, _col(mvh, 0), _col(mvh, 0))
            nc.vector.tensor_add(t_f, t_f, _col(mvh, 1))
            nc.vector.tensor_scalar_mul(out=_col(pk2, 1), in0=t_f, scalar1=float(POS))
            ps_s2 = ps.tile([1, 2], F32, tag="ps", name="ps_s2")
            nc.tensor.matmul(ps_s2, ones, pk2, start=True, stop=True)
            d2o = dram.tile([8], F32, name="d2o")
            row2 = stile([1, 2], "row2")
            nc.vector.tensor_copy(out=row2, in_=ps_s2)
            nc.sync.dma_start(out=d2i[0:2], in_=row2)
            nc.gpsimd.collective_compute(
                "AllReduce", mybir.AluOpType.add,
                replica_groups=[list(range(N_CORES))],
                ins=[d2i.opt()], outs=[d2o.opt()])
            g2 = bcast_readback(d2o, 2, "g2")

            jnk2 = ps.tile([1, 512], F32, tag="ps", name="jnk2")
            for n in range(10):
                nc.tensor.matmul(jnk2, al1b, x_sb[:, bass.ts(n % 8, 512)],
                                 start=True, stop=True)

            mu2, r2 = gn_mu_r(g2, 0, 1, N1, "2")
            al2 = stile([128, 1], "al2")
            nc.vector.tensor_mul(al2, r2, _col(pp, 4))
            be2 = stile([128, 1], "be2")
            nc.vector.tensor_mul(be2, mu2, al2)
            nc.vector.tensor_sub(be2, _col(pp, 5), be2)

            m_col = stile([128, 1], "m_col")
            nc.scalar.activation(out=h2b, in_=h2,
                                 func=AF.Gelu, bias=be2, scale=al2,
                                 accum_out=m_col)
            # preload the erf/sigmoid table during the AR3 window (the SE
            # gelu runs via erf so the whole SE block needs ONE table)
            dummy2 = stile([1, 1], "dummy2")
            nc.vector.tensor_copy(out=dummy2, in_=m_col[0:1, :])
            nc.scalar.activation(out=dummy2, in_=dummy2, func=AF.Erf)
            nc.sync.dma_start(out=djunk, in_=dummy2)
            ps_t = ps.tile([1, 128], F32, tag="ps", name="ps_t")
            nc.tensor.matmul(ps_t, m_col, id_sb, start=True, stop=True)
            m_row = stile([1, 128], "m_row")
            nc.vector.tensor_copy(out=m_row, in_=ps_t)
            d3i = dram.tile([128], F32, name="d3i")
            d3o = dram.tile([128], F32, name="d3o")
            nc.sync.dma_start(out=d3i, in_=m_row)
            nc.gpsimd.collective_compute(
                "AllReduce", mybir.AluOpType.add,
                replica_groups=[list(range(N_CORES))],
                ins=[d3i.opt()], outs=[d3o.opt()])
            m_row2 = stile([1, 128], "m_row2")
            nc.sync.dma_start(out=m_row2, in_=d3o)
            ps_mt = ps.tile([128, 1], F32, tag="ps", name="ps_mt")
            nc.tensor.matmul(ps_mt, m_row2, one_t, start=True, stop=True)
            m_sb = ps_mt

            m_mean = stile([128, 1], "m_mean")
            nc.vector.tensor_scalar_mul(out=m_mean, in0=m_sb, scalar1=1.0 / P_SP)
            ps_se1 = ps.tile([8, 1], F32, tag="ps", name="ps_se1")
            nc.tensor.matmul(ps_se1, pp[:, 16:24], m_mean, start=True, stop=True)
            # gelu(y) = 0.5*y*(1+erf(y/sqrt2)) -- erf is in the sigmoid table
            y1e = stile([8, 1], "y1e")
            nc.scalar.activation(out=y1e, in_=ps_se1, func=AF.Erf,
                                 scale=0.70710678)
            y1g = stile([8, 1], "y1g")
            nc.vector.tensor_mul(y1g[0:8], ps_se1, y1e[0:8])
            nc.vector.tensor_add(y1g[0:8], y1g[0:8], ps_se1)
            nc.vector.tensor_scalar_mul(out=y1g[0:8], in0=y1g[0:8], scalar1=0.5)
            ps_se2 = ps.tile([128, 1], F32, tag="ps", name="ps_se2")
            nc.tensor.matmul(ps_se2, pp[0:8, 56:184], y1g[0:8], start=True, stop=True)
            s_sb = stile([128, 1], "s_sb")
            nc.scalar.activation(out=s_sb, in_=ps_se2, func=AF.Sigmoid)
            w3s = small.tile([128, 32], BF16, name="w3s")
            nc.vector.tensor_scalar_mul(out=w3s, in0=pp[:, 24:56], scalar1=s_sb)

            y3p = big.tile([96, 6 * 512], BF16, name="y3p", tag="h1slot")
            st3 = stile([96, 6, 6], "st3")
            nc.vector.memset(st3, 0.0)
            for g in range(6):
                nj = 3 if g < 5 else 1
                pt3 = ps.tile([96, 512], F32, tag="ps", name=f"c3_{g}")
                for j in range(nj):
                    n = 3 * g + j
                    nc.tensor.matmul(pt3[32 * j:32 * j + 32, :], w3s,
                                     h2b[:, bass.ts(n, 512)],
                                     start=True, stop=True)
                nc.vector.bn_stats(out=st3[0:32 * nj, g, :],
                                   in_=pt3[0:32 * nj, :])
                blk = bass.ts(g, 512)
                nc.scalar.copy(out=y3p[0:32 * nj, blk], in_=pt3[0:32 * nj, :])
            mv3 = stile([96, 2], "mv3")
            nc.vector.bn_aggr(out=mv3, in_=st3)
            pk3 = stile([128, 2], "pk3")
            nc.vector.memset(pk3, 0.0)
            # per-partition sample counts (3072 for band 0, 2560 for bands
            # 1,2 which skip group 5) live in pp col 15
            nc.vector.tensor_mul(pk3[0:96, 0:1], mv3[:, 0:1], pp[0:96, 15:16])
            t_g = sc("t_g")
            nc.vector.tensor_mul(t_g[0:96], mv3[:, 0:1], mv3[:, 0:1])
            nc.vector.tensor_add(t_g[0:96], t_g[0:96], mv3[:, 1:2])
            nc.vector.tensor_mul(pk3[0:96, 1:2], t_g[0:96], pp[0:96, 15:16])
            ps_s3 = ps.tile([1, 2], F32, tag="ps", name="ps_s3")
            nc.tensor.matmul(ps_s3, ones, pk3, start=True, stop=True)
            d4o = dram.tile([8], F32, name="d4o")
            row3 = stile([1, 2], "row3")
            nc.vector.tensor_copy(out=row3, in_=ps_s3)
            nc.sync.dma_start(out=d4i[0:2], in_=row3)
            nc.gpsimd.collective_compute(
                "AllReduce", mybir.AluOpType.add,
                replica_groups=[list(range(N_CORES))],
                ins=[d4i.opt()], outs=[d4o.opt()])
            g4 = bcast_readback(d4o, 2, "g4")

            mu3, r3 = gn_mu_r(g4, 0, 1, N3, "3")
            al3 = stile([128, 1], "al3")
            nc.vector.tensor_mul(al3, r3, _col(pp, 6))
            be3 = stile([128, 1], "be3")
            nc.vector.tensor_mul(be3, mu3, al3)
            nc.vector.tensor_sub(be3, _col(pp, 7), be3)

            y3f = big.tile([96, 6 * 512], F32, name="y3f", tag="apslot")
            for half in range(2):
                cs = half * 3 * 512
                nc.vector.tensor_scalar(out=y3f[:, cs:cs + 3 * 512],
                                        in0=y3p[0:96, cs:cs + 3 * 512],
                                        scalar1=al3[0:96], scalar2=be3[0:96],
                                        op0=mybir.AluOpType.mult,
                                        op1=mybir.AluOpType.add)
                g_lo = half * 3
                for j in range(3):
                    ng = 3 if (half == 0 or j == 0) else 2
                    sb_ap = y3f[32 * j:32 * j + 32].rearrange(
                        "p (g i) -> p g i", g=6, i=512)[:, g_lo:g_lo + ng, :]
                    dram_ap = bass.AP(
                        tensor=out_d.tensor,
                        offset=out_d.offset + 512 * j + 3 * 512 * g_lo,
                        ap=[[POS, 32], [3 * 512, ng], [1, 512]])
                    nc.sync.dma_start(out=dram_ap, in_=sb_ap)

    nc.compile()
    return nc


def _host_prep(inputs):
    x = np.asarray(inputs['x'], np.float32).reshape(CIN, S, S, S, S)
    g0w = np.asarray(inputs['g0_w'], np.float32)
    g0b = np.asarray(inputs['g0_b'], np.float32)
    W1 = np.asarray(inputs['w1'], np.float32).reshape(HID, CIN)
    gn1w = np.asarray(inputs['gn1_w'], np.float32)
    gn1b = np.asarray(inputs['gn1_b'], np.float32)
    w2 = np.asarray(inputs['w2'], np.float32).reshape(HID, HID, 3, 3, 3, 3)
    gn2w = np.asarray(inputs['gn2_w'], np.float32)
    gn2b = np.asarray(inputs['gn2_b'], np.float32)
    se1 = np.asarray(inputs['se_w1'], np.float32)
    se2 = np.asarray(inputs['se_w2'], np.float32)
    W3 = np.asarray(inputs['w3'], np.float32).reshape(CIN, HID)
    gn3w = np.asarray(inputs['gn3_w'], np.float32)
    gn3b = np.asarray(inputs['gn3_b'], np.float32)

    w1fold = W1 * g0w[None, :]
    w1rep = np.zeros((128, 128), np.float32)
    for j in range(4):
        w1rep[32 * j:32 * j + 32, :] = w1fold.T
    w1rep = w1rep.astype(ml_dtypes.bfloat16)
    u = W1 @ g0b
    v = W1 @ g0w
    w2t = np.ascontiguousarray(
        w2.transpose(1, 2, 3, 4, 5, 0).reshape(HID, 81 * HID)).astype(
            ml_dtypes.bfloat16)

    params = np.zeros((128, 192), np.float32)
    params[:, 0] = u
    params[:, 1] = v
    params[:, 2] = gn1w
    params[:, 3] = gn1b
    params[:, 4] = gn2w
    params[:, 5] = gn2b
    params[0:96, 6] = np.tile(gn3w, 3)
    params[0:96, 7] = np.tile(gn3b, 3)
    params[:, 10] = u.sum()
    params[:, 11] = v.sum()
    params[:, 12] = (u * u).sum()
    params[:, 13] = (u * v).sum()
    params[:, 14] = (v * v).sum()
    params[0:32, 15] = 6.0 * 512.0
    params[32:96, 15] = 5.0 * 512.0
    params[:, 16:24] = se1.T
    params[:, 24:56] = W3.T
    params[0:8, 56:184] = se2.T

    xp = np.zeros((CIN, S + 2, S, S, S), np.float32)
    xp[:, 1:S + 1] = x

    in_maps = []
    for k in range(N_CORES):
        p = params.copy()
        p[:, 8] = 0.0 if k == 0 else 1.0
        p[:, 9] = 0.0 if k == N_CORES - 1 else 1.0
        idx = [2 * k + 1, 2 * k + 2, 2 * k, 2 * k + 3]
        shard = np.ascontiguousarray(
            xp[:, idx].transpose(1, 0, 2, 3, 4).reshape(128, PLANE)).astype(
                ml_dtypes.bfloat16)
        in_maps.append({"xs": shard, "w1rep": w1rep, "w2t": w2t, "params": p,
                        "ident": np.eye(128, dtype=np.float32)})
    return in_maps


def kernel(**inputs):
    if "nc" not in _cache:
        _cache["nc"] = build_program()
    nc = _cache["nc"]
    in_maps = _host_prep(inputs)
    res = run_bass_kernel_spmd(nc, in_maps, core_ids=list(range(N_CORES)))
    out = np.empty((1, CIN, S, S, S, S), np.float32)
    for k in range(N_CORES):
        out[0, :, 2 * k:2 * k + 2] = res.results[k]["out"].reshape(CIN, 2, S, S, S)
    return out


def run_traced(inputs):
    if "nc" not in _cache:
        _cache["nc"] = build_program()
    nc = _cache["nc"]
    in_maps = _host_prep(inputs)
    res = run_bass_kernel_spmd(nc, in_maps, core_ids=list(range(N_CORES)),
                               trace=True)
    out = np.empty((1, CIN, S, S, S, S), np.float32)
    for k in range(N_CORES):
        out[0, :, 2 * k:2 * k + 2] = res.results[k]["out"].reshape(CIN, 2, S, S, S)
    return out, res
